# revision 40
# baseline (speedup 1.0000x reference)
"""Trainium2 Bass kernel for nn_Attention_83004537963197.

LayerNorm -> QKV projection -> 8-head attention (head_dim=16) -> output
projection, x[16, 1024, 1024] f32.  Data-parallel over batch: 2 batches
per NeuronCore across 8 cores, no collectives.

v2 changes vs baseline (301us):
  * x is shipped bf16 from the host (halves input DMA, 2x faster LN ops).
  * exp is split across ScalarE (exact, activation Exp) and VectorE
    (Schraudolph bit-hack: round(s*128*log2e + 16250.5) as int16 IS the
    bf16 pattern of ~e^s, one tensor_scalar op; ~3% elementwise, cancels
    through the softmax normalization).
  * LN rsqrt = exp(-0.5*ln(var+eps)) so every ScalarE activation stays in
    the natural_log_exp table set - the baseline paid ~14us of
    ACT_TABLE_LOAD thrash between sqrt and exp.
  * softmax normalize: one PSUM->SBUF copy of the whole [P,2,512] oT
    (rowsum rows included), DRAM-bounce reshape to [128,32] for the
    reciprocal, DRAM-bounce partition-broadcast back, one fused mul.
  * projection accumulates both regions in PSUM (no SBUF stash+add).
  * batch-0 prep-phase PSUM->SBUF copies run on the then-idle ScalarE.

Per-core dataflow (per batch):
  A. Load x row tiles [128, 1024] bf16, LayerNorm along free dim
     (bn_stats), normalize to bf16, transpose via PE matmul against a
     constant identity.
  B. q^T/k^T compact [128(f), n] via matmul with gamma/SCALE-folded
     weights, then SBUF->SBUF DMA relocation of each head's 16 rows to
     32-aligned "region" layout (4 heads per region at offsets 32c).
     v in row layout per (j-tile, head) as [128, 32]: col 0 = 1.0
     (softmax rowsum trick), cols 1..16 = v, rest 0.
  C. Per (r, ih, cp, jt): scores S^T[j,i] = k_h^T.T @ q_h^T (K=16,
     row-tiled via tile_position), exp (ScalarE or DVE per schedule),
     attn@v as oT[d,i] += v_aug.T @ E^T (K=128, col-tiled).  The ones
     column gives softmax row sums at oT row 32c; normalize as above.
     Row 32c becomes exactly 1.0; region 0 row 0 pairs with b_proj in
     w_proj_pad row 0 to add the bias for free.
  D. Projection with zero-padded w_proj rows.

Emission is software-pipelined across the 2 batches: batch b+1's
LN/qkv/v chunks and batch b's projection chunks are emitted between
attention groups of the current batch.
"""

from contextlib import ExitStack

import numpy as np
import ml_dtypes

import concourse.bass as bass
import concourse.tile as tile
from concourse import bacc, mybir
from concourse.bass_utils import run_bass_kernel_spmd

F32 = mybir.dt.float32
BF16 = mybir.dt.bfloat16
I16 = mybir.dt.int16

B, N, EMB = 16, 1024, 1024
HEADS, INNER = 8, 128
HD = INNER // HEADS            # 16
SCALE = INNER ** -0.5
EPS = 1e-5
NCORES = 8
NB = B // NCORES               # batches per core
P = 128
NT = EMB // P                  # 8 tiles along emb / n

Sub = mybir.AluOpType.subtract
Mult = mybir.AluOpType.mult
Add = mybir.AluOpType.add
AF = mybir.ActivationFunctionType

K1 = 128 * 1.4426950408889634        # schraudolph scale
K2 = 16256.0 - 5.5                   # schraudolph bias (HW rounds to nearest)
RSQRT_MAGIC = 0x5f3759df
I32 = mybir.dt.int32

_CACHE = {}


def _dve_tile(b, r, ih, jt, cp):
    """Which exp tiles go to the DVE (Schraudolph bit-hack exp).

    Both engines run ~1.1-1.2us per [P, 1024] tile; the DVE also carries
    the LN/normalize work, so it gets ~31% of the exp tiles (which also
    bounds the Schraudolph contribution to the output error).
    """
    return cp == 0 and (jt % 2 == 0 or jt == 1)


def _build():
    nc = bacc.Bacc(None, target_bir_lowering=False)

    xs_h = nc.declare_dram_parameter("xs", [NB, N, EMB], BF16, isOutput=False)
    wqk_h = nc.declare_dram_parameter("wqk", [P, NT, 2, P], BF16, isOutput=False)
    bqk_h = nc.declare_dram_parameter("bqk", [P, 2], F32, isOutput=False)
    wv_h = nc.declare_dram_parameter("wv", [P, NT, P], BF16, isOutput=False)
    bv_h = nc.declare_dram_parameter("bv", [1, P], BF16, isOutput=False)
    wpj_h = nc.declare_dram_parameter("wproj", [P, 2, EMB], BF16, isOutput=False)
    id_h = nc.declare_dram_parameter("ident", [P, P], BF16, isOutput=False)
    out_h = nc.declare_dram_parameter("out", [NB, N, EMB], F32, isOutput=True)

    with tile.TileContext(nc) as tc, ExitStack() as ctx:
        ent = ctx.enter_context
        const = ent(tc.tile_pool(name="const", bufs=1))
        xpool = ent(tc.tile_pool(name="xpool", bufs=10))
        stat = ent(tc.tile_pool(name="stat", bufs=8))
        xT_pool = ent(tc.tile_pool(name="xT", bufs=2))
        qk_pool = ent(tc.tile_pool(name="qk", bufs=2))
        v_pool = ent(tc.tile_pool(name="vp", bufs=2))
        e_pool = ent(tc.tile_pool(name="ep", bufs=4))
        o_pool = ent(tc.tile_pool(name="op", bufs=4))
        nrm_pool = ent(tc.tile_pool(name="nrm", bufs=2))
        fin_pool = ent(tc.tile_pool(name="fin", bufs=4))
        dram_pool = ent(tc.tile_pool(name="dsc", bufs=2, space="DRAM"))
        ps_small = ent(tc.tile_pool(name="pss", bufs=1, space="PSUM"))
        ps_sc = ent(tc.tile_pool(name="psc", bufs=3, space="PSUM"))
        ps_oT = ent(tc.tile_pool(name="pso", bufs=1, space="PSUM"))

        # ---- constants ----
        wqk_sb = const.tile([P, NT, 2, P], BF16)
        nc.sync.dma_start(out=wqk_sb, in_=wqk_h[:])
        bqk_sb = const.tile([P, 2], F32)
        nc.sync.dma_start(out=bqk_sb, in_=bqk_h[:])
        wv_sb = const.tile([P, NT, P], BF16)
        nc.sync.dma_start(out=wv_sb, in_=wv_h[:])
        bv_sb = const.tile([1, P], BF16)
        nc.sync.dma_start(out=bv_sb, in_=bv_h[:])
        wpj_sb = const.tile([P, 2, EMB], BF16)
        nc.sync.dma_start(out=wpj_sb, in_=wpj_h[:])
        id_sb = const.tile([P, P], BF16)
        nc.sync.dma_start(out=id_sb, in_=id_h[:])
        eps_sb = const.tile([P, 1], F32)
        nc.vector.memset(eps_sb, EPS)
        ones1_sb = const.tile([1, P], BF16)
        nc.vector.memset(ones1_sb, 1.0)

        st8 = {0: {}, 1: {}}   # per-batch live tiles

        def emit_stat(b, it):
            s = st8[b]
            if s.get("xT") is None:
                s["xT"] = xT_pool.tile([P, NT, N], BF16, tag="xTt",
                                       name=f"xT{b}")
                s["xn"] = [None] * NT
                s["xt"] = [None] * NT
                s["mv"] = stat.tile([P, NT, 2], F32, tag="mvall",
                                    name=f"mv{b}")
                s["rs"] = [None, None]
            xt = xpool.tile([P, EMB], BF16, tag="xt")
            nc.sync.dma_start(out=xt, in_=xs_h[b, it * P:(it + 1) * P, :])
            st = stat.tile([P, 2, 6], F32, tag="st")
            nc.vector.bn_stats(out=st[:, 0, :], in_=xt[:, 0:512])
            nc.vector.bn_stats(out=st[:, 1, :], in_=xt[:, 512:1024])
            nc.vector.bn_aggr(out=s["mv"][:, it, :], in_=st)
            s["xt"][it] = xt

        def emit_rsqrt(b, h):
            # rs[4h..4h+3] = 1/sqrt(var+eps) entirely on the DVE:
            # fast-inverse-sqrt bit hack + 2 Newton steps, [P, 4] wide.
            s = st8[b]
            var = s["mv"][:, 4 * h:4 * h + 4, 1:2]
            ve = stat.tile([P, 4], F32, tag="ve")
            nc.vector.tensor_scalar(out=ve, in0=var, scalar1=EPS,
                                    scalar2=None, op0=Add)
            iv = stat.tile([P, 4], I32, tag="iv")
            nc.vector.tensor_scalar(out=iv, in0=ve[:].bitcast(I32),
                                    scalar1=1, scalar2=None,
                                    op0=mybir.AluOpType.arith_shift_right)
            y0 = stat.tile([P, 4], I32, tag="y0")
            nc.vector.tensor_scalar(out=y0, in0=iv, scalar1=-1,
                                    scalar2=float(RSQRT_MAGIC),
                                    op0=Mult, op1=Add)
            y = y0[:].bitcast(F32)
            for itn in range(1):
                t = stat.tile([P, 4], F32, tag=f"nt{itn}")
                nc.vector.tensor_tensor(out=t, in0=y, in1=y, op=Mult)
                nc.vector.tensor_tensor(out=t, in0=t, in1=ve, op=Mult)
                nc.vector.tensor_scalar(out=t, in0=t, scalar1=-0.5,
                                        scalar2=1.5, op0=Mult, op1=Add)
                yn = stat.tile([P, 4], F32, tag=f"ny{itn}")
                nc.vector.tensor_tensor(out=yn, in0=y, in1=t, op=Mult)
                y = yn
            s["rs"][h] = y

        def emit_ln_apply(b, it):
            s = st8[b]
            xt = s["xt"][it]
            rs = s["rs"][it // 4][:, (it % 4):(it % 4) + 1]
            xn = xpool.tile([P, EMB], BF16, tag="xn")
            nc.vector.tensor_scalar(
                out=xn, in0=xt, scalar1=s["mv"][:, it, 0:1], scalar2=rs,
                op0=Sub, op1=Mult)
            s["xn"][it] = xn
            s["xt"][it] = None

        def emit_tp(b, it, ceng="s"):
            s = st8[b]
            xT = s["xT"]
            xn = s["xn"][it]
            for eg in range(2):
                tp = ps_small.tile([P, 4, P], F32, tag="smallps")
                for kk in range(4):
                    et = 4 * eg + kk
                    nc.tensor.matmul(
                        tp[:, kk, :], xn[:, et * P:(et + 1) * P], id_sb,
                        start=True, stop=True)
                dst = xT[:, 4 * eg:4 * eg + 4, it * P:(it + 1) * P]
                if ceng == "s":
                    nc.scalar.copy(out=dst, in_=tp)
                else:
                    nc.vector.tensor_copy(out=dst, in_=tp)

        def emit_qk(b, t, nt):
            # compact q^T/k^T halves; on the last nt of each t, relocate
            # head rows into the 32-aligned region layout.
            s = st8[b]
            if s.get("qkc") is None:
                s["qkc"] = qk_pool.tile([P, 2, N], BF16, tag="qkc",
                                        name=f"qkc{b}")
                s["qT"] = qk_pool.tile([P, 2, N], BF16, tag="qT",
                                       name=f"qT{b}")
                s["kT"] = qk_pool.tile([P, 2, N], BF16, tag="kT",
                                       name=f"kT{b}")
            xT = s["xT"]
            ps = ps_oT.tile([P, 512], F32, tag="oTps")
            for et in range(NT):
                nc.tensor.matmul(
                    ps, wqk_sb[:, et, t, :],
                    xT[:, et, nt * 512:(nt + 1) * 512],
                    start=(et == 0), stop=(et == NT - 1))
            nc.vector.tensor_scalar(
                out=s["qkc"][:, t, nt * 512:(nt + 1) * 512], in0=ps,
                scalar1=bqk_sb[:, t:t + 1], scalar2=None, op0=Add)
            if nt == 1:
                # relocation DMAs ride the (otherwise idle) GPSIMD SWDGE
                # ring: they can wait a long time for qkc, and a parked
                # DMA head-of-line-blocks every later DMA on its ring.
                dst = s["qT"] if t == 0 else s["kT"]
                for h in range(HEADS):
                    r, c = h // 4, h % 4
                    nc.gpsimd.dma_start(
                        out=dst[32 * c:32 * c + HD, r, :],
                        in_=s["qkc"][HD * h:HD * (h + 1), t, :])

        def emit_v(b, jt, ceng="s"):
            s = st8[b]
            if s.get("v") is None:
                s["v"] = v_pool.tile([P, NT, HEADS, 32], BF16, tag="vt",
                                     name=f"v{b}")
                nc.gpsimd.memset(s["v"], 0.0)
                nc.gpsimd.memset(s["v"][:, :, :, 0:1], 1.0)
            xT = s["xT"]
            ps = ps_oT.tile([P, P], F32, tag="oTps")
            for et in range(NT):
                nc.tensor.matmul(
                    ps, xT[:, et, jt * P:(jt + 1) * P], wv_sb[:, et, :],
                    start=(et == 0), stop=False)
            nc.tensor.matmul(ps, ones1_sb, bv_sb, start=False, stop=True)
            src = ps[:].rearrange("p (h d) -> p h d", d=16)
            if ceng == "s":
                nc.scalar.copy(out=s["v"][:, jt, :, 1:17], in_=src)
            else:
                nc.vector.tensor_copy(out=s["v"][:, jt, :, 1:17], in_=src)

        def emit_proj(b, it, nt, ceng):
            s = st8[b]
            ps = ps_small.tile([P, 512], F32, tag="smallps")
            for r in range(2):
                nc.tensor.matmul(
                    ps, s["o"][r][:, it * P:(it + 1) * P],
                    wpj_sb[:, r, nt * 512:(nt + 1) * 512],
                    start=(r == 0), stop=(r == 1))
            fin = fin_pool.tile([P, 512], F32, tag="fin")
            if ceng == "s":
                nc.scalar.copy(out=fin, in_=ps)
            else:
                nc.vector.tensor_copy(out=fin, in_=ps)
            nc.sync.dma_start(
                out=out_h[b, it * P:(it + 1) * P, nt * 512:(nt + 1) * 512],
                in_=fin)

        def emit_normalize(b, r, ih, oT_ps):
            # oT_ps [P, 512] f32: rows 32c = rowsums, rows 32c+1+d = head
            # (4r+c) outputs.  Divide every row of band c by the band's
            # rowsum (per free position).
            s = st8[b]
            if s["o"][r] is None:
                s["o"][r] = o_pool.tile([P, N], BF16, tag="oT",
                                        name=f"o{b}{r}")
            t_sb = nrm_pool.tile([P, 512], BF16, tag="tsb")
            nc.vector.tensor_copy(out=t_sb, in_=oT_ps)
            scr1 = dram_pool.tile([4, 512], BF16, tag="scr1")
            nc.sync.dma_start(out=scr1, in_=t_sb[0::32, :])
            cmp = nrm_pool.tile([P, 16], BF16, tag="cmp")
            flat = scr1[:].rearrange("c (pp cc) -> (c pp) cc", cc=16)
            nc.sync.dma_start(out=cmp, in_=flat)
            rec = nrm_pool.tile([P, 16], BF16, tag="rec")
            with nc.allow_low_precision(reason="bf16 softmax rowsum recip"):
                nc.vector.reciprocal(out=rec, in_=cmp)
            scr2 = dram_pool.tile([4, 512], BF16, tag="scr2")
            nc.sync.dma_start(
                out=scr2[:].rearrange("c (pp cc) -> (c pp) cc", cc=16),
                in_=rec)
            rep = nrm_pool.tile([P, 512], BF16, tag="rep")
            for c in range(4):
                src = scr2[c:c + 1, :]
                bcast = bass.AP(
                    tensor=src.tensor, offset=src.offset,
                    ap=[[0, 32]] + list(src.ap[1:]))
                nc.sync.dma_start(
                    out=rep[32 * c:32 * c + 32, :], in_=bcast)
            i0 = ih * 512
            nc.vector.tensor_tensor(
                out=s["o"][r][:, i0:i0 + 512],
                in0=t_sb, in1=rep, op=Mult)

        def emit_proj1(b, it, nt, ceng):
            # region-0 half of the projection, stashed in SBUF bf16
            s = st8[b]
            if s.get("fin1") is None:
                s["fin1"] = fin_pool.tile([P, NT, 2, 512], BF16,
                                          tag="fin1", name="fin1", bufs=1)
            ps = ps_small.tile([P, 512], F32, tag="smallps")
            nc.tensor.matmul(
                ps, s["o"][0][:, it * P:(it + 1) * P],
                wpj_sb[:, 0, nt * 512:(nt + 1) * 512],
                start=True, stop=True)
            if ceng == "s":
                nc.scalar.copy(out=s["fin1"][:, it, nt, :], in_=ps)
            else:
                nc.vector.tensor_copy(out=s["fin1"][:, it, nt, :], in_=ps)

        def emit_proj2(b, it, nt):
            s = st8[b]
            ps = ps_small.tile([P, 512], F32, tag="smallps")
            nc.tensor.matmul(
                ps, s["o"][1][:, it * P:(it + 1) * P],
                wpj_sb[:, 1, nt * 512:(nt + 1) * 512],
                start=True, stop=True)
            fin = fin_pool.tile([P, 512], F32, tag="fin")
            nc.vector.tensor_add(fin, s["fin1"][:, it, nt, :], ps)
            nc.sync.dma_start(
                out=out_h[b, it * P:(it + 1) * P, nt * 512:(nt + 1) * 512],
                in_=fin)

        def emit_attention(b, fillers, rate=2):
            # 4-band groups: all four head-bands' score matmuls run
            # concurrently in the PE (row-tiled), the two band-pairs' exps
            # run concurrently on ScalarE and VectorE (Schraudolph), and
            # the four attn@v matmuls run concurrently (col-tiled).
            # Software-pipelined by one group so the in-order PE queue
            # never head-of-line-blocks on an exp result.
            s = st8[b]
            s["o"] = [None, None]
            slot = [0]

            def maybe_fill():
                slot[0] += 1
                if fillers and slot[0] % rate == 0:
                    f = fillers.pop(0)
                    if f is not None:
                        f()

            def scores_exp(r, ih, jt):
                i0 = ih * 512
                E = e_pool.tile([P, 4, 512], BF16, tag="E")
                scs = []
                for cp in range(2):
                    sc = ps_sc.tile([P, 2, 512], F32, tag="sc")
                    scs.append(sc)
                    for ci in range(2):
                        c = 2 * cp + ci
                        nc.tensor.matmul(
                            sc[:, ci, :],
                            s["kT"][32 * c:32 * c + 16, r,
                                    jt * P:(jt + 1) * P],
                            s["qT"][32 * c:32 * c + 16, r, i0:i0 + 512],
                            start=True, stop=True,
                            tile_position=(32 * c, 0))
                for cp in range(2):
                    dst = E[:, 2 * cp:2 * cp + 2, :]
                    if _dve_tile(b, r, ih, jt, cp):
                        nc.vector.tensor_scalar(
                            out=dst.bitcast(I16), in0=scs[cp],
                            scalar1=K1, scalar2=K2, op0=Mult, op1=Add)
                    else:
                        nc.scalar.activation(out=dst, in_=scs[cp],
                                             func=AF.Exp)
                return E

            def attnv(r, ih, jt, E, oT_ps):
                for c in range(4):
                    h = 4 * r + c
                    nc.tensor.matmul(
                        oT_ps[32 * c:32 * c + 32, :],
                        s["v"][:, jt, h, :], E[:, c, :],
                        start=(jt == 0), stop=(jt == NT - 1),
                        tile_position=(0, 32 * c))

            from collections import deque
            pend = deque()

            def retire():
                g = pend.popleft()
                attnv(*g)
                if g[2] == NT - 1:   # last group of (r, ih)
                    emit_normalize(b, g[0], g[1], g[4])

            # ih-major block order: after the first two blocks both
            # regions' ih=0 halves are normalized, so projection chunks
            # for the first half of the sequence can run as fillers.
            for ih in range(2):
                for r in range(2):
                    oT_ps = ps_oT.tile([P, 512], F32, tag="oTps")
                    for jt in range(NT):
                        E = scores_exp(r, ih, jt)
                        pend.append((r, ih, jt, E, oT_ps))
                        if len(pend) > 1:
                            retire()
                        maybe_fill()
            while pend:
                retire()

        # ---------- schedule ----------
        # preload the exp table set while the DMA ramp runs
        dummy = stat.tile([P, 1], F32, tag="dummy")
        nc.scalar.activation(out=dummy, in_=eps_sb, func=AF.Exp)

        def ab_order(b):
            # v(it) lags tp(it) by two chunks so the in-order PE queue
            # always has independent transpose work behind a v-chunk that
            # is still waiting on its xT copy.
            o = []
            for it in range(4):
                o.append(lambda it=it: emit_stat(b, it))
            o.append(lambda: emit_rsqrt(b, 0))
            o.append(lambda: emit_ln_apply(b, 0))
            o.append(lambda: emit_ln_apply(b, 1))
            o.append(lambda: emit_tp(b, 0))
            o.append(lambda: emit_stat(b, 4))
            o.append(lambda: emit_ln_apply(b, 2))
            o.append(lambda: emit_tp(b, 1))
            o.append(lambda: emit_stat(b, 5))
            o.append(lambda: emit_ln_apply(b, 3))
            o.append(lambda: emit_tp(b, 2))
            o.append(lambda: emit_stat(b, 6))
            o.append(lambda: emit_v(b, 0))
            o.append(lambda: emit_tp(b, 3))
            o.append(lambda: emit_stat(b, 7))
            o.append(lambda: emit_v(b, 1))
            o.append(lambda: emit_qk(b, 0, 0))
            o.append(lambda: emit_qk(b, 1, 0))
            o.append(lambda: emit_rsqrt(b, 1))
            o.append(lambda: emit_ln_apply(b, 4))
            o.append(lambda: emit_tp(b, 4))
            o.append(lambda: emit_v(b, 2))
            o.append(lambda: emit_ln_apply(b, 5))
            o.append(lambda: emit_tp(b, 5))
            o.append(lambda: emit_v(b, 3))
            o.append(lambda: emit_ln_apply(b, 6))
            o.append(lambda: emit_tp(b, 6))
            o.append(lambda: emit_v(b, 4))
            o.append(lambda: emit_ln_apply(b, 7))
            o.append(lambda: emit_tp(b, 7, "v"))
            o.append(lambda: emit_v(b, 5, "v"))
            o.append(lambda: emit_v(b, 6, "v"))
            o.append(lambda: emit_v(b, 7, "v"))
            o.append(lambda: emit_qk(b, 0, 1))
            o.append(lambda: emit_qk(b, 1, 1))
            return o

        # Phase A: both batches' LN/transpose/qkv prep, interleaved with a
        # two-chunk offset so same-kind chunks (which share a PSUM ring)
        # are never adjacent.  ScalarE only does PSUM->SBUF copies here,
        # so the DVE (stats, rsqrt, LN apply) and PE (transposes, qkv
        # matmuls) pipeline.  Batch 1's last prep chunks spill into
        # phase B0 as fillers so B0's exp stream starts sooner.
        a0, a1 = ab_order(0), ab_order(1)
        a1_tail = a1[-5:]
        a1 = a1[:-5]
        a0[0](); a0[1]()
        for i in range(len(a0)):
            if i + 2 < len(a0):
                a0[i + 2]()
            if i < len(a1):
                a1[i]()

        # Phase B0: attention for batch 0; batch 1's prep tail as
        # sparse fillers.
        emit_attention(0, a1_tail, rate=5)
        for f in a1_tail:
            f()

        # Phase B1: attention for batch 1.  Batch 0's projection fills
        # the first half; batch 1's first-sequence-half projection
        # chunks can start once both regions' ih=0 blocks are
        # normalized (slots 17+).
        fill = [lambda it=it, nt=nt: emit_proj(0, it, nt, "v")
                for it in range(NT) for nt in range(2)]
        fill += [None] * 2
        fill += [lambda it=it, nt=nt: emit_proj(1, it, nt, "v")
                 for it in range(4) for nt in range(2)]
        emit_attention(1, fill, rate=1)
        for f in fill:
            if f is not None:
                f()

        for it in range(4, NT):
            for nt in range(2):
                emit_proj(1, it, nt, "s" if (it + nt) % 2 else "v")

    nc.finalize()
    return nc


def _prep_weights(gamma, beta, w_qkv, w_proj, b_proj):
    gamma = gamma.astype(np.float64)
    beta = beta.astype(np.float64)
    w_qkv = w_qkv.astype(np.float64)
    w_proj = w_proj.astype(np.float64)
    b_proj = b_proj.astype(np.float64)

    wg = w_qkv * gamma[:, None]
    bias = beta @ w_qkv                   # [384]

    # compact q/k: tile t=0 -> q (SCALE folded), t=1 -> k
    wqk = np.zeros((EMB, 2, P), dtype=np.float64)
    wqk[:, 0, :] = wg[:, :INNER] * SCALE
    wqk[:, 1, :] = wg[:, INNER:2 * INNER]
    bqk = np.zeros((P, 2), dtype=np.float64)
    bqk[:, 0] = bias[:INNER] * SCALE
    bqk[:, 1] = bias[INNER:2 * INNER]
    wqk_t = wqk.reshape(NT, P, 2, P).transpose(1, 0, 2, 3)  # [P, NT, 2, P]

    wv = wg[:, 2 * INNER:3 * INNER].reshape(NT, P, P).transpose(1, 0, 2)
    bv = bias[2 * INNER:3 * INNER].reshape(1, P)

    # o^T row mapping: 32c = ones/rowsum row, 32c+1+d = head (4r+c) dim d
    wpj = np.zeros((P, 2, EMB), dtype=np.float64)
    for r in range(2):
        for c in range(4):
            h = 4 * r + c
            wpj[32 * c + 1:32 * c + 1 + HD, r, :] = \
                w_proj[h * HD:(h + 1) * HD, :]
    wpj[0, 0, :] = b_proj

    bf = ml_dtypes.bfloat16
    return {
        "wqk": np.ascontiguousarray(wqk_t).astype(bf),
        "bqk": np.ascontiguousarray(bqk).astype(np.float32),
        "wv": np.ascontiguousarray(wv).astype(bf),
        "bv": np.ascontiguousarray(bv).astype(bf),
        "wproj": np.ascontiguousarray(wpj).astype(bf),
        "ident": np.eye(P, dtype=np.float32).astype(bf),
    }


def kernel(x, gamma, beta, w_qkv, w_proj, b_proj):
    if "nc" not in _CACHE:
        _CACHE["nc"] = _build()
    nc = _CACHE["nc"]

    w = _prep_weights(gamma, beta, w_qkv, w_proj, b_proj)
    xb = np.asarray(x, dtype=np.float32).astype(ml_dtypes.bfloat16)
    in_maps = []
    for i in range(NCORES):
        m = {"xs": np.ascontiguousarray(xb[i * NB:(i + 1) * NB])}
        m.update(w)
        in_maps.append(m)

    res = run_bass_kernel_spmd(nc, in_maps, core_ids=list(range(NCORES)))
    out = np.concatenate([res.results[i]["out"] for i in range(NCORES)], axis=0)
    return out.astype(np.float32)


# revision 41
# speedup vs baseline: 1.0302x; 1.0302x over previous
"""Trainium2 Bass kernel for nn_Attention_83004537963197.

LayerNorm -> QKV projection -> 8-head attention (head_dim=16) -> output
projection, x[16, 1024, 1024] f32.  Data-parallel over batch: 2 batches
per NeuronCore across 8 cores, no collectives.

v2 changes vs baseline (301us):
  * x is shipped bf16 from the host (halves input DMA, 2x faster LN ops).
  * exp is split across ScalarE (exact, activation Exp) and VectorE
    (Schraudolph bit-hack: round(s*128*log2e + 16250.5) as int16 IS the
    bf16 pattern of ~e^s, one tensor_scalar op; ~3% elementwise, cancels
    through the softmax normalization).
  * LN rsqrt = exp(-0.5*ln(var+eps)) so every ScalarE activation stays in
    the natural_log_exp table set - the baseline paid ~14us of
    ACT_TABLE_LOAD thrash between sqrt and exp.
  * softmax normalize: one PSUM->SBUF copy of the whole [P,2,512] oT
    (rowsum rows included), DRAM-bounce reshape to [128,32] for the
    reciprocal, DRAM-bounce partition-broadcast back, one fused mul.
  * projection accumulates both regions in PSUM (no SBUF stash+add).
  * batch-0 prep-phase PSUM->SBUF copies run on the then-idle ScalarE.

Per-core dataflow (per batch):
  A. Load x row tiles [128, 1024] bf16, LayerNorm along free dim
     (bn_stats), normalize to bf16, transpose via PE matmul against a
     constant identity.
  B. q^T/k^T compact [128(f), n] via matmul with gamma/SCALE-folded
     weights, then SBUF->SBUF DMA relocation of each head's 16 rows to
     32-aligned "region" layout (4 heads per region at offsets 32c).
     v in row layout per (j-tile, head) as [128, 32]: col 0 = 1.0
     (softmax rowsum trick), cols 1..16 = v, rest 0.
  C. Per (r, ih, cp, jt): scores S^T[j,i] = k_h^T.T @ q_h^T (K=16,
     row-tiled via tile_position), exp (ScalarE or DVE per schedule),
     attn@v as oT[d,i] += v_aug.T @ E^T (K=128, col-tiled).  The ones
     column gives softmax row sums at oT row 32c; normalize as above.
     Row 32c becomes exactly 1.0; region 0 row 0 pairs with b_proj in
     w_proj_pad row 0 to add the bias for free.
  D. Projection with zero-padded w_proj rows.

Emission is software-pipelined across the 2 batches: batch b+1's
LN/qkv/v chunks and batch b's projection chunks are emitted between
attention groups of the current batch.
"""

from contextlib import ExitStack

import numpy as np
import ml_dtypes

import concourse.bass as bass
import concourse.tile as tile
from concourse import bacc, mybir
from concourse.bass_utils import run_bass_kernel_spmd

F32 = mybir.dt.float32
BF16 = mybir.dt.bfloat16
I16 = mybir.dt.int16

B, N, EMB = 16, 1024, 1024
HEADS, INNER = 8, 128
HD = INNER // HEADS            # 16
SCALE = INNER ** -0.5
EPS = 1e-5
NCORES = 8
NB = B // NCORES               # batches per core
P = 128
NT = EMB // P                  # 8 tiles along emb / n

Sub = mybir.AluOpType.subtract
Mult = mybir.AluOpType.mult
Add = mybir.AluOpType.add
AF = mybir.ActivationFunctionType

K1 = 128 * 1.4426950408889634        # schraudolph scale
K2 = 16256.0 - 5.5                   # schraudolph bias (HW rounds to nearest)
RSQRT_MAGIC = 0x5f3759df
I32 = mybir.dt.int32

_CACHE = {}


def _dve_tile(b, r, ih, jt, cp):
    """Which exp tiles go to the DVE (Schraudolph bit-hack exp).

    Both engines run ~1.1-1.2us per [P, 1024] tile; the DVE also carries
    the LN/normalize work, so it gets ~31% of the exp tiles (which also
    bounds the Schraudolph contribution to the output error).
    """
    return cp == 0 and (jt % 2 == 0 or jt in (1, 5))


def _build():
    nc = bacc.Bacc(None, target_bir_lowering=False)

    xs_h = nc.declare_dram_parameter("xs", [NB, N, EMB], BF16, isOutput=False)
    wqk_h = nc.declare_dram_parameter("wqk", [P, NT, 2, P], BF16, isOutput=False)
    bqk_h = nc.declare_dram_parameter("bqk", [P, 2], F32, isOutput=False)
    wv_h = nc.declare_dram_parameter("wv", [P, NT, P], BF16, isOutput=False)
    bv_h = nc.declare_dram_parameter("bv", [1, P], BF16, isOutput=False)
    wpj_h = nc.declare_dram_parameter("wproj", [P, 2, EMB], BF16, isOutput=False)
    id_h = nc.declare_dram_parameter("ident", [P, P], BF16, isOutput=False)
    out_h = nc.declare_dram_parameter("out", [NB, N, EMB], F32, isOutput=True)

    with tile.TileContext(nc) as tc, ExitStack() as ctx:
        ent = ctx.enter_context
        const = ent(tc.tile_pool(name="const", bufs=1))
        xpool = ent(tc.tile_pool(name="xpool", bufs=10))
        stat = ent(tc.tile_pool(name="stat", bufs=8))
        xT_pool = ent(tc.tile_pool(name="xT", bufs=2))
        qk_pool = ent(tc.tile_pool(name="qk", bufs=2))
        v_pool = ent(tc.tile_pool(name="vp", bufs=2))
        e_pool = ent(tc.tile_pool(name="ep", bufs=4))
        o_pool = ent(tc.tile_pool(name="op", bufs=4))
        nrm_pool = ent(tc.tile_pool(name="nrm", bufs=2))
        fin_pool = ent(tc.tile_pool(name="fin", bufs=4))
        dram_pool = ent(tc.tile_pool(name="dsc", bufs=2, space="DRAM"))
        ps_small = ent(tc.tile_pool(name="pss", bufs=1, space="PSUM"))
        ps_sc = ent(tc.tile_pool(name="psc", bufs=3, space="PSUM"))
        ps_oT = ent(tc.tile_pool(name="pso", bufs=1, space="PSUM"))

        # ---- constants ----
        wqk_sb = const.tile([P, NT, 2, P], BF16)
        nc.sync.dma_start(out=wqk_sb, in_=wqk_h[:])
        bqk_sb = const.tile([P, 2], F32)
        nc.sync.dma_start(out=bqk_sb, in_=bqk_h[:])
        wv_sb = const.tile([P, NT, P], BF16)
        nc.sync.dma_start(out=wv_sb, in_=wv_h[:])
        bv_sb = const.tile([1, P], BF16)
        nc.sync.dma_start(out=bv_sb, in_=bv_h[:])
        wpj_sb = const.tile([P, 2, EMB], BF16)
        nc.sync.dma_start(out=wpj_sb, in_=wpj_h[:])
        id_sb = const.tile([P, P], BF16)
        nc.sync.dma_start(out=id_sb, in_=id_h[:])
        eps_sb = const.tile([P, 1], F32)
        nc.vector.memset(eps_sb, EPS)
        ones1_sb = const.tile([1, P], BF16)
        nc.vector.memset(ones1_sb, 1.0)

        st8 = {0: {}, 1: {}}   # per-batch live tiles

        def emit_stat(b, it):
            s = st8[b]
            if s.get("xT") is None:
                s["xT"] = xT_pool.tile([P, NT, N], BF16, tag="xTt",
                                       name=f"xT{b}")
                s["xn"] = [None] * NT
                s["xt"] = [None] * NT
                s["mv"] = stat.tile([P, NT, 2], F32, tag="mvall",
                                    name=f"mv{b}")
                s["rs"] = [None, None]
            xt = xpool.tile([P, EMB], BF16, tag="xt")
            nc.sync.dma_start(out=xt, in_=xs_h[b, it * P:(it + 1) * P, :])
            st = stat.tile([P, 2, 6], F32, tag="st")
            nc.vector.bn_stats(out=st[:, 0, :], in_=xt[:, 0:512])
            nc.vector.bn_stats(out=st[:, 1, :], in_=xt[:, 512:1024])
            nc.vector.bn_aggr(out=s["mv"][:, it, :], in_=st)
            s["xt"][it] = xt

        def emit_rsqrt(b, h):
            # rs[4h..4h+3] = 1/sqrt(var+eps) entirely on the DVE:
            # fast-inverse-sqrt bit hack + 2 Newton steps, [P, 4] wide.
            s = st8[b]
            var = s["mv"][:, 4 * h:4 * h + 4, 1:2]
            ve = stat.tile([P, 4], F32, tag="ve")
            nc.vector.tensor_scalar(out=ve, in0=var, scalar1=EPS,
                                    scalar2=None, op0=Add)
            iv = stat.tile([P, 4], I32, tag="iv")
            nc.vector.tensor_scalar(out=iv, in0=ve[:].bitcast(I32),
                                    scalar1=1, scalar2=None,
                                    op0=mybir.AluOpType.arith_shift_right)
            y0 = stat.tile([P, 4], I32, tag="y0")
            nc.vector.tensor_scalar(out=y0, in0=iv, scalar1=-1,
                                    scalar2=float(RSQRT_MAGIC),
                                    op0=Mult, op1=Add)
            y = y0[:].bitcast(F32)
            for itn in range(1):
                t = stat.tile([P, 4], F32, tag=f"nt{itn}")
                nc.vector.tensor_tensor(out=t, in0=y, in1=y, op=Mult)
                nc.vector.tensor_tensor(out=t, in0=t, in1=ve, op=Mult)
                nc.vector.tensor_scalar(out=t, in0=t, scalar1=-0.5,
                                        scalar2=1.5, op0=Mult, op1=Add)
                yn = stat.tile([P, 4], F32, tag=f"ny{itn}")
                nc.vector.tensor_tensor(out=yn, in0=y, in1=t, op=Mult)
                y = yn
            s["rs"][h] = y

        def emit_ln_apply(b, it):
            s = st8[b]
            xt = s["xt"][it]
            rs = s["rs"][it // 4][:, (it % 4):(it % 4) + 1]
            xn = xpool.tile([P, EMB], BF16, tag="xn")
            nc.vector.tensor_scalar(
                out=xn, in0=xt, scalar1=s["mv"][:, it, 0:1], scalar2=rs,
                op0=Sub, op1=Mult)
            s["xn"][it] = xn
            s["xt"][it] = None

        def emit_tp(b, it, ceng="s"):
            s = st8[b]
            xT = s["xT"]
            xn = s["xn"][it]
            for eg in range(2):
                tp = ps_small.tile([P, 4, P], F32, tag="smallps")
                for kk in range(4):
                    et = 4 * eg + kk
                    nc.tensor.matmul(
                        tp[:, kk, :], xn[:, et * P:(et + 1) * P], id_sb,
                        start=True, stop=True)
                dst = xT[:, 4 * eg:4 * eg + 4, it * P:(it + 1) * P]
                if ceng == "s":
                    nc.scalar.copy(out=dst, in_=tp)
                else:
                    nc.vector.tensor_copy(out=dst, in_=tp)

        def emit_qk(b, t, nt):
            # compact q^T/k^T halves; on the last nt of each t, relocate
            # head rows into the 32-aligned region layout.
            s = st8[b]
            if s.get("qkc") is None:
                s["qkc"] = qk_pool.tile([P, 2, N], BF16, tag="qkc",
                                        name=f"qkc{b}")
                s["qT"] = qk_pool.tile([P, 2, N], BF16, tag="qT",
                                       name=f"qT{b}")
                s["kT"] = qk_pool.tile([P, 2, N], BF16, tag="kT",
                                       name=f"kT{b}")
            xT = s["xT"]
            ps = ps_oT.tile([P, 512], F32, tag="oTps")
            for et in range(NT):
                nc.tensor.matmul(
                    ps, wqk_sb[:, et, t, :],
                    xT[:, et, nt * 512:(nt + 1) * 512],
                    start=(et == 0), stop=(et == NT - 1))
            nc.vector.tensor_scalar(
                out=s["qkc"][:, t, nt * 512:(nt + 1) * 512], in0=ps,
                scalar1=bqk_sb[:, t:t + 1], scalar2=None, op0=Add)
            if nt == 1:
                # relocation DMAs ride the (otherwise idle) GPSIMD SWDGE
                # ring: they can wait a long time for qkc, and a parked
                # DMA head-of-line-blocks every later DMA on its ring.
                dst = s["qT"] if t == 0 else s["kT"]
                for h in range(HEADS):
                    r, c = h // 4, h % 4
                    nc.gpsimd.dma_start(
                        out=dst[32 * c:32 * c + HD, r, :],
                        in_=s["qkc"][HD * h:HD * (h + 1), t, :])

        def emit_v(b, jt, ceng="s"):
            s = st8[b]
            if s.get("v") is None:
                s["v"] = v_pool.tile([P, NT, HEADS, 32], BF16, tag="vt",
                                     name=f"v{b}")
                nc.gpsimd.memset(s["v"], 0.0)
                nc.gpsimd.memset(s["v"][:, :, :, 0:1], 1.0)
            xT = s["xT"]
            ps = ps_oT.tile([P, P], F32, tag="oTps")
            for et in range(NT):
                nc.tensor.matmul(
                    ps, xT[:, et, jt * P:(jt + 1) * P], wv_sb[:, et, :],
                    start=(et == 0), stop=False)
            nc.tensor.matmul(ps, ones1_sb, bv_sb, start=False, stop=True)
            src = ps[:].rearrange("p (h d) -> p h d", d=16)
            if ceng == "s":
                nc.scalar.copy(out=s["v"][:, jt, :, 1:17], in_=src)
            else:
                nc.vector.tensor_copy(out=s["v"][:, jt, :, 1:17], in_=src)

        def emit_proj(b, it, nt, ceng, ring=0):
            s = st8[b]
            pool = ps_small if ring == 0 else ps_oT
            tagn = "smallps" if ring == 0 else "oTps"
            ps = pool.tile([P, 512], F32, tag=tagn)
            for r in range(2):
                nc.tensor.matmul(
                    ps, s["o"][r][:, it * P:(it + 1) * P],
                    wpj_sb[:, r, nt * 512:(nt + 1) * 512],
                    start=(r == 0), stop=(r == 1))
            fin = fin_pool.tile([P, 512], F32, tag="fin")
            if ceng == "s":
                nc.scalar.copy(out=fin, in_=ps)
            else:
                nc.vector.tensor_copy(out=fin, in_=ps)
            nc.sync.dma_start(
                out=out_h[b, it * P:(it + 1) * P, nt * 512:(nt + 1) * 512],
                in_=fin)

        def emit_normalize(b, r, ih, oT_ps):
            # oT_ps [P, 512] f32: rows 32c = rowsums, rows 32c+1+d = head
            # (4r+c) outputs.  Divide every row of band c by the band's
            # rowsum (per free position).
            s = st8[b]
            if s["o"][r] is None:
                s["o"][r] = o_pool.tile([P, N], BF16, tag="oT",
                                        name=f"o{b}{r}")
            t_sb = nrm_pool.tile([P, 512], BF16, tag="tsb")
            nc.vector.tensor_copy(out=t_sb, in_=oT_ps)
            # the bounce chain rides the GPSIMD SWDGE ring so its parked
            # DMAs never head-of-line-block the SP ring's output stream
            scr1 = dram_pool.tile([4, 512], BF16, tag="scr1")
            nc.gpsimd.dma_start(out=scr1, in_=t_sb[0::32, :])
            cmp = nrm_pool.tile([P, 16], BF16, tag="cmp")
            flat = scr1[:].rearrange("c (pp cc) -> (c pp) cc", cc=16)
            nc.gpsimd.dma_start(out=cmp, in_=flat)
            rec = nrm_pool.tile([P, 16], BF16, tag="rec")
            with nc.allow_low_precision(reason="bf16 softmax rowsum recip"):
                nc.vector.reciprocal(out=rec, in_=cmp)
            scr2 = dram_pool.tile([4, 512], BF16, tag="scr2")
            nc.gpsimd.dma_start(
                out=scr2[:].rearrange("c (pp cc) -> (c pp) cc", cc=16),
                in_=rec)
            rep = nrm_pool.tile([P, 512], BF16, tag="rep")
            for c in range(4):
                src = scr2[c:c + 1, :]
                bcast = bass.AP(
                    tensor=src.tensor, offset=src.offset,
                    ap=[[0, 32]] + list(src.ap[1:]))
                nc.gpsimd.dma_start(
                    out=rep[32 * c:32 * c + 32, :], in_=bcast)
            i0 = ih * 512
            nc.vector.tensor_tensor(
                out=s["o"][r][:, i0:i0 + 512],
                in0=t_sb, in1=rep, op=Mult)

        def emit_proj1(b, it, nt, ceng):
            # region-0 half of the projection, stashed in SBUF bf16
            s = st8[b]
            if s.get("fin1") is None:
                s["fin1"] = fin_pool.tile([P, NT, 2, 512], BF16,
                                          tag="fin1", name="fin1", bufs=1)
            ps = ps_small.tile([P, 512], F32, tag="smallps")
            nc.tensor.matmul(
                ps, s["o"][0][:, it * P:(it + 1) * P],
                wpj_sb[:, 0, nt * 512:(nt + 1) * 512],
                start=True, stop=True)
            if ceng == "s":
                nc.scalar.copy(out=s["fin1"][:, it, nt, :], in_=ps)
            else:
                nc.vector.tensor_copy(out=s["fin1"][:, it, nt, :], in_=ps)

        def emit_proj2(b, it, nt):
            s = st8[b]
            ps = ps_small.tile([P, 512], F32, tag="smallps")
            nc.tensor.matmul(
                ps, s["o"][1][:, it * P:(it + 1) * P],
                wpj_sb[:, 1, nt * 512:(nt + 1) * 512],
                start=True, stop=True)
            fin = fin_pool.tile([P, 512], F32, tag="fin")
            nc.vector.tensor_add(fin, s["fin1"][:, it, nt, :], ps)
            nc.sync.dma_start(
                out=out_h[b, it * P:(it + 1) * P, nt * 512:(nt + 1) * 512],
                in_=fin)

        def emit_attention(b, fillers, rate=2):
            # 4-band groups: all four head-bands' score matmuls run
            # concurrently in the PE (row-tiled), the two band-pairs' exps
            # run concurrently on ScalarE and VectorE (Schraudolph), and
            # the four attn@v matmuls run concurrently (col-tiled).
            # Software-pipelined by one group so the in-order PE queue
            # never head-of-line-blocks on an exp result.
            s = st8[b]
            s["o"] = [None, None]
            slot = [0]

            def maybe_fill():
                slot[0] += 1
                if fillers and slot[0] % rate == 0:
                    f = fillers.pop(0)
                    if f is not None:
                        f()

            def scores_exp(r, ih, jt):
                i0 = ih * 512
                E = e_pool.tile([P, 4, 512], BF16, tag="E")
                scs = []
                for cp in range(2):
                    sc = ps_sc.tile([P, 2, 512], F32, tag="sc")
                    scs.append(sc)
                    for ci in range(2):
                        c = 2 * cp + ci
                        nc.tensor.matmul(
                            sc[:, ci, :],
                            s["kT"][32 * c:32 * c + 16, r,
                                    jt * P:(jt + 1) * P],
                            s["qT"][32 * c:32 * c + 16, r, i0:i0 + 512],
                            start=True, stop=True,
                            tile_position=(32 * c, 0))
                for cp in range(2):
                    dst = E[:, 2 * cp:2 * cp + 2, :]
                    if _dve_tile(b, r, ih, jt, cp):
                        nc.vector.tensor_scalar(
                            out=dst.bitcast(I16), in0=scs[cp],
                            scalar1=K1, scalar2=K2, op0=Mult, op1=Add)
                    else:
                        nc.scalar.activation(out=dst, in_=scs[cp],
                                             func=AF.Exp)
                return E

            def attnv(r, ih, jt, E, oT_ps):
                for c in range(4):
                    h = 4 * r + c
                    nc.tensor.matmul(
                        oT_ps[32 * c:32 * c + 32, :],
                        s["v"][:, jt, h, :], E[:, c, :],
                        start=(jt == 0), stop=(jt == NT - 1),
                        tile_position=(0, 32 * c))

            from collections import deque
            pend = deque()

            def retire():
                g = pend.popleft()
                attnv(*g)
                if g[2] == NT - 1:   # last group of (r, ih)
                    emit_normalize(b, g[0], g[1], g[4])

            # ih-major block order: after the first two blocks both
            # regions' ih=0 halves are normalized, so projection chunks
            # for the first half of the sequence can run as fillers.
            for ih in range(2):
                for r in range(2):
                    oT_ps = ps_oT.tile([P, 512], F32, tag="oTps")
                    for jt in range(NT):
                        E = scores_exp(r, ih, jt)
                        pend.append((r, ih, jt, E, oT_ps))
                        if len(pend) > 1:
                            retire()
                        maybe_fill()
            while pend:
                retire()

        # ---------- schedule ----------
        # preload the exp table set while the DMA ramp runs
        dummy = stat.tile([P, 1], F32, tag="dummy")
        nc.scalar.activation(out=dummy, in_=eps_sb, func=AF.Exp)

        def ab_order(b):
            # v(it) lags tp(it) by two chunks so the in-order PE queue
            # always has independent transpose work behind a v-chunk that
            # is still waiting on its xT copy.
            o = []
            for it in range(4):
                o.append(lambda it=it: emit_stat(b, it))
            o.append(lambda: emit_rsqrt(b, 0))
            o.append(lambda: emit_ln_apply(b, 0))
            o.append(lambda: emit_ln_apply(b, 1))
            o.append(lambda: emit_tp(b, 0))
            o.append(lambda: emit_stat(b, 4))
            o.append(lambda: emit_ln_apply(b, 2))
            o.append(lambda: emit_tp(b, 1))
            o.append(lambda: emit_stat(b, 5))
            o.append(lambda: emit_ln_apply(b, 3))
            o.append(lambda: emit_tp(b, 2))
            o.append(lambda: emit_stat(b, 6))
            o.append(lambda: emit_v(b, 0))
            o.append(lambda: emit_tp(b, 3))
            o.append(lambda: emit_stat(b, 7))
            o.append(lambda: emit_v(b, 1))
            o.append(lambda: emit_qk(b, 0, 0))
            o.append(lambda: emit_qk(b, 1, 0))
            o.append(lambda: emit_rsqrt(b, 1))
            o.append(lambda: emit_ln_apply(b, 4))
            o.append(lambda: emit_tp(b, 4))
            o.append(lambda: emit_v(b, 2))
            o.append(lambda: emit_ln_apply(b, 5))
            o.append(lambda: emit_tp(b, 5))
            o.append(lambda: emit_v(b, 3))
            o.append(lambda: emit_ln_apply(b, 6))
            o.append(lambda: emit_tp(b, 6))
            o.append(lambda: emit_v(b, 4))
            o.append(lambda: emit_ln_apply(b, 7))
            o.append(lambda: emit_tp(b, 7))
            o.append(lambda: emit_qk(b, 0, 1))
            o.append(lambda: emit_qk(b, 1, 1))
            o.append(lambda: emit_v(b, 5, "v"))
            o.append(lambda: emit_v(b, 6, "v"))
            o.append(lambda: emit_v(b, 7, "v"))
            return o

        # Phase A: both batches' LN/transpose/qkv prep, interleaved with a
        # two-chunk offset so same-kind chunks (which share a PSUM ring)
        # are never adjacent.  ScalarE only does PSUM->SBUF copies here,
        # so the DVE (stats, rsqrt, LN apply) and PE (transposes, qkv
        # matmuls) pipeline.  Batch 1's last prep chunks spill into
        # phase B0 as fillers so B0's exp stream starts sooner.
        a0, a1 = ab_order(0), ab_order(1)
        a1_tail = a1[-5:]
        a1 = a1[:-5]
        a0[0](); a0[1]()
        for i in range(len(a0)):
            if i + 2 < len(a0):
                a0[i + 2]()
            if i < len(a1):
                a1[i]()

        # Phase B0: attention for batch 0; batch 1's prep tail as
        # sparse fillers.
        emit_attention(0, a1_tail, rate=5)
        for f in a1_tail:
            f()

        # Phase B1: attention for batch 1.  Batch 0's projection fills
        # the first half; batch 1's first-sequence-half projection
        # chunks can start once both regions' ih=0 blocks are
        # normalized (slots 17+).
        fill = [lambda it=it, nt=nt: emit_proj(0, it, nt, "v")
                for it in range(NT) for nt in range(2)]
        fill += [None] * 2
        fill += [lambda it=it, nt=nt: emit_proj(1, it, nt, "v")
                 for it in range(4) for nt in range(2)]
        emit_attention(1, fill, rate=1)
        for f in fill:
            if f is not None:
                f()

        k = 0
        for it in range(4, NT):
            for nt in range(2):
                emit_proj(1, it, nt, "s" if k % 2 else "v", ring=k % 2)
                k += 1

    nc.finalize()
    return nc


def _prep_weights(gamma, beta, w_qkv, w_proj, b_proj):
    gamma = gamma.astype(np.float64)
    beta = beta.astype(np.float64)
    w_qkv = w_qkv.astype(np.float64)
    w_proj = w_proj.astype(np.float64)
    b_proj = b_proj.astype(np.float64)

    wg = w_qkv * gamma[:, None]
    bias = beta @ w_qkv                   # [384]

    # compact q/k: tile t=0 -> q (SCALE folded), t=1 -> k
    wqk = np.zeros((EMB, 2, P), dtype=np.float64)
    wqk[:, 0, :] = wg[:, :INNER] * SCALE
    wqk[:, 1, :] = wg[:, INNER:2 * INNER]
    bqk = np.zeros((P, 2), dtype=np.float64)
    bqk[:, 0] = bias[:INNER] * SCALE
    bqk[:, 1] = bias[INNER:2 * INNER]
    wqk_t = wqk.reshape(NT, P, 2, P).transpose(1, 0, 2, 3)  # [P, NT, 2, P]

    wv = wg[:, 2 * INNER:3 * INNER].reshape(NT, P, P).transpose(1, 0, 2)
    bv = bias[2 * INNER:3 * INNER].reshape(1, P)

    # o^T row mapping: 32c = ones/rowsum row, 32c+1+d = head (4r+c) dim d
    wpj = np.zeros((P, 2, EMB), dtype=np.float64)
    for r in range(2):
        for c in range(4):
            h = 4 * r + c
            wpj[32 * c + 1:32 * c + 1 + HD, r, :] = \
                w_proj[h * HD:(h + 1) * HD, :]
    wpj[0, 0, :] = b_proj

    bf = ml_dtypes.bfloat16
    return {
        "wqk": np.ascontiguousarray(wqk_t).astype(bf),
        "bqk": np.ascontiguousarray(bqk).astype(np.float32),
        "wv": np.ascontiguousarray(wv).astype(bf),
        "bv": np.ascontiguousarray(bv).astype(bf),
        "wproj": np.ascontiguousarray(wpj).astype(bf),
        "ident": np.eye(P, dtype=np.float32).astype(bf),
    }


def kernel(x, gamma, beta, w_qkv, w_proj, b_proj):
    if "nc" not in _CACHE:
        _CACHE["nc"] = _build()
    nc = _CACHE["nc"]

    w = _prep_weights(gamma, beta, w_qkv, w_proj, b_proj)
    xb = np.asarray(x, dtype=np.float32).astype(ml_dtypes.bfloat16)
    in_maps = []
    for i in range(NCORES):
        m = {"xs": np.ascontiguousarray(xb[i * NB:(i + 1) * NB])}
        m.update(w)
        in_maps.append(m)

    res = run_bass_kernel_spmd(nc, in_maps, core_ids=list(range(NCORES)))
    out = np.concatenate([res.results[i]["out"] for i in range(NCORES)], axis=0)
    return out.astype(np.float32)
